# revision 52
# baseline (speedup 1.0000x reference)
"""AdvancedMuonAttention Trainium2 kernel (8 NeuronCores, SPMD, no collectives).

Sharding: core c -> (batch b = c//2, query half q = c%2).  Each core computes
its [1024, 1024] slice of the output (including RMSNorm) entirely locally:
q-projection on its 1024 query rows, k/v-projections on the full 2048 keys of
its batch (duplicated across the 2 cores sharing a batch), attention, output
projection, RMSNorm.  The host shards inputs / reassembles outputs.

Device-side layout choices (validated by probes):
  - activations channels-first [D, S]; weights pre-transposed [D_in, D_out]
  - the per-head NeuralAttention transform is folded into wq/wk on the host
    (W' = blockdiag(na_w) @ W, b' = blockdiag(na_w) @ b + tile(na_b)), so
    the q/k projections emit tanh(...) directly from the projection PSUM
  - fp32r (fp32 rounded to 11 mantissa bits, full PE speed) for projections
  - bf16 for qn/kn/P/mask/v (2x DVE modes); fp32 PSUM accumulation
  - scoresT [k, q] orientation: softmax sums ride the ctx matmul via a ones
    column appended to v (M=65); division by sums is applied to ctx
  - exp without max subtraction (scores are bounded); masking = multiply
    exp(scores) by {0,1} mask

Schedule (v2): the Act engine's 256 exp instructions (~294us at 100% duty)
are the kernel floor, so the program is ordered to saturate Act as early as
possible and keep it saturated: K proj -> Q proj (tanh warms the exp table
set) -> V proj interleaved per-k-tile with attention (j=0, qb=0) -> rest of
attention with wo/rmsw prefetched underneath -> output projection + RMSNorm.
"""
import sys
import numpy as np
import ml_dtypes

sys.path.insert(0, "/opt/trn_rl_repo")

import concourse.bacc as bacc
import concourse.mybir as mybir
import concourse.tile as tile
from concourse.bass_utils import run_bass_kernel_spmd

F32 = mybir.dt.float32
F32R = mybir.dt.float32r
BF16 = mybir.dt.bfloat16

B, S, D, H, DK = 4, 2048, 1024, 16, 64
SQ = 1024            # query rows per core
P = 128              # partitions
NCORES = 8
NKT = S // P         # 16 k-tiles
NJ = H // 2          # 8 head pairs / d-block pairs
EPS = 1e-8


def _f32r_round(x):
    """RNE-round fp32 to 11 mantissa bits (the PE's fp32r operand format)."""
    u = np.ascontiguousarray(x, dtype=np.float32).view(np.uint32)
    r = ((u.astype(np.uint64) + 0x7FF + ((u >> 12) & 1)) & 0xFFFFF000).astype(np.uint32)
    return r.view(np.float32)


def build_nc():
    nc = bacc.Bacc("TRN2", target_bir_lowering=False)

    # inputs ----------------------------------------------------------------
    qt_in = nc.declare_dram_parameter("qt_in", [D, SQ], BF16, isOutput=False)
    kt_in = nc.declare_dram_parameter("kt_in", [D, S], BF16, isOutput=False)
    vt_in = nc.declare_dram_parameter("vt_in", [D, S], BF16, isOutput=False)
    maskt = nc.declare_dram_parameter("maskt", [S, SQ], BF16, isOutput=False)
    wqt = nc.declare_dram_parameter("wqt", [D, D], BF16, isOutput=False)
    wkt = nc.declare_dram_parameter("wkt", [D, D], BF16, isOutput=False)
    wvt = nc.declare_dram_parameter("wvt", [D, D], BF16, isOutput=False)
    wot = nc.declare_dram_parameter("wot", [D, D], BF16, isOutput=False)
    bqt = nc.declare_dram_parameter("bqt", [P, 8], F32, isOutput=False)
    bkt = nc.declare_dram_parameter("bkt", [P, 8], F32, isOutput=False)
    bvr = nc.declare_dram_parameter("bvr", [1, D], F32R, isOutput=False)
    bor = nc.declare_dram_parameter("bor", [1, D], F32R, isOutput=False)
    tscp = nc.declare_dram_parameter("tscp", [P, 8], F32, isOutput=False)
    rmsw = nc.declare_dram_parameter("rmsw", [P, D], BF16, isOutput=False)
    onesr = nc.declare_dram_parameter("onesr", [1, P], F32R, isOutput=False)
    out = nc.declare_dram_parameter("out", [SQ, D], F32, isOutput=True)

    sums_d = nc.dram_tensor("sums_d", [H, SQ], F32)
    recip_d = nc.dram_tensor("recip_d", [H, SQ], F32)

    AF = mybir.ActivationFunctionType
    OP = mybir.AluOpType

    with tile.TileContext(nc) as tc:
        import contextlib
        es = contextlib.ExitStack()
        with es:
            # long-lived pools
            const = es.enter_context(tc.tile_pool(name="const", bufs=1))
            qcp = es.enter_context(tc.tile_pool(name="qcp", bufs=9))
            wrk = es.enter_context(tc.tile_pool(name="wrk", bufs=1))
            wop = es.enter_context(tc.tile_pool(name="wop", bufs=1))
            es2 = es.enter_context(contextlib.ExitStack())
            knp = es2.enter_context(tc.tile_pool(name="knp", bufs=1))
            vap = es2.enter_context(tc.tile_pool(name="vap", bufs=1))

            # constants
            onesr_sb = const.tile([1, P], F32R, name="onesr_sb")
            nc.sync.dma_start(out=onesr_sb[:, :], in_=onesr[:, :])
            bvr_sb = const.tile([1, D], F32R, name="bvr_sb")
            nc.sync.dma_start(out=bvr_sb[:, :], in_=bvr[:, :])
            bor_sb = const.tile([1, D], F32R, name="bor_sb")
            nc.sync.dma_start(out=bor_sb[:, :], in_=bor[:, :])
            bqt_sb = const.tile([P, 8], F32, name="bqt_sb")
            nc.sync.dma_start(out=bqt_sb[:, :], in_=bqt[:, :])
            bkt_sb = const.tile([P, 8], F32, name="bkt_sb")
            nc.sync.dma_start(out=bkt_sb[:, :], in_=bkt[:, :])
            tscp_sb = const.tile([P, 8], F32, name="tscp_sb")
            nc.sync.dma_start(out=tscp_sb[:, :], in_=tscp[:, :])

            # long-lived tensors.  v is split by head-half (ob): heads 0-7
            # feed attention j=0..3, heads 8-15 feed j=4..7 — this lets the
            # ob1 half of the V projection run underneath Act-saturated
            # attention instead of blocking it.
            knt = [knp.tile([P, S], BF16, tag=f"kn{j}", name=f"knt{j}")
                   for j in range(NJ)]
            vau = [[vap.tile([P, 8, DK + 1], BF16, tag=f"v{ob}_{st}",
                             name=f"vaug{ob}_{st}") for st in range(NKT)]
                   for ob in range(2)]
            qn = [qcp.tile([P, SQ], BF16, tag="qc", name=f"qn{j}")
                  for j in range(NJ)]
            for ob in range(2):
                for st in range(NKT):
                    nc.vector.memset(vau[ob][st][:, :, DK:DK + 1], 1.0)

            # transient input pool for K/Q chunks (freed before attention)
            eskq = contextlib.ExitStack()
            inp = eskq.enter_context(tc.tile_pool(name="inp", bufs=2))
            wqp = eskq.enter_context(tc.tile_pool(name="wqp", bufs=1))
            wq_t = []

            # ---------------- phase K: kn = tanh(K @ (naK@wk).T + b') -------
            with tc.tile_pool(name="wkp", bufs=1) as wkp, \
                 tc.tile_pool(name="kqprj", bufs=4, space="PSUM") as kqprj:
                wk_t = []
                for ib in range(8):
                    w_t = wkp.tile([P, D], BF16, tag=f"wk{ib}", name=f"wk{ib}")
                    nc.sync.dma_start(out=w_t[:, :], in_=wkt[ib * P:(ib + 1) * P, :])
                    wk_t.append(w_t)
                for sbi in range(4):
                    kin_t = []
                    for ib in range(8):
                        t = inp.tile([P, 512], BF16, tag=f"in{ib}", name=f"kin{ib}_{sbi}")
                        nc.sync.dma_start(
                            out=t[:, :],
                            in_=kt_in[ib * P:(ib + 1) * P, sbi * 512:(sbi + 1) * 512])
                        kin_t.append(t)
                    if sbi == 0:
                        # prefetch the Q weights under the K projection
                        for ib in range(8):
                            w_t = wqp.tile([P, D], BF16, tag=f"wq{ib}",
                                           name=f"wq{ib}")
                            nc.sync.dma_start(
                                out=w_t[:, :],
                                in_=wqt[ib * P:(ib + 1) * P, :])
                            wq_t.append(w_t)
                    for j in range(NJ):
                        pk = kqprj.tile([P, 512], F32, tag="prj", name=f"pk{sbi}_{j}")
                        for ib in range(8):
                            nc.tensor.matmul(
                                pk[:, :],
                                lhsT=wk_t[ib][:, j * P:(j + 1) * P],
                                rhs=kin_t[ib][:, :],
                                start=(ib == 0), stop=(ib == 7))
                        nc.scalar.activation(
                            knt[j][:, sbi * 512:(sbi + 1) * 512], pk[:, :],
                            AF.Tanh, bias=bkt_sb[:, j:j + 1])

            # ---------------- phase Q ---------------------------------------
            with tc.tile_pool(name="qprj", bufs=4, space="PSUM") as kqprj:
                if True:
                    for sbi in range(2):
                        qin_t = []
                        for ib in range(8):
                            t = inp.tile([P, 512], BF16, tag=f"in{ib}", name=f"qin{ib}_{sbi}")
                            nc.sync.dma_start(
                                out=t[:, :],
                                in_=qt_in[ib * P:(ib + 1) * P, sbi * 512:(sbi + 1) * 512])
                            qin_t.append(t)
                        for j in range(NJ):
                            pq = kqprj.tile([P, 512], F32, tag="prj", name=f"pq{sbi}_{j}")
                            for ib in range(8):
                                nc.tensor.matmul(
                                    pq[:, :],
                                    lhsT=wq_t[ib][:, j * P:(j + 1) * P],
                                    rhs=qin_t[ib][:, :],
                                    start=(ib == 0), stop=(ib == 7))
                            nc.scalar.activation(
                                qn[j][:, sbi * 512:(sbi + 1) * 512], pq[:, :],
                                AF.Tanh, bias=bqt_sb[:, j:j + 1])
                    # fold 1/(sqrt(DK)*temp_h) into qn
                    for j in range(NJ):
                        nc.vector.tensor_scalar_mul(qn[j][:, :], qn[j][:, :],
                                                    tscp_sb[:, j:j + 1])

            eskq.close()   # free the K/Q input pool

            # ---------------- V proj + attention (overlapped) ---------------
            ctx = []
            esa = contextlib.ExitStack()
            maskp = esa.enter_context(tc.tile_pool(name="maskp", bufs=1))
            ppool = esa.enter_context(tc.tile_pool(name="pp", bufs=4))
            psc = esa.enter_context(tc.tile_pool(name="psc", bufs=2, space="PSUM"))

            esp1 = contextlib.ExitStack()
            pss = esp1.enter_context(tc.tile_pool(name="pss1", bufs=1,
                                                  space="PSUM"))

            esv = contextlib.ExitStack()
            wvp = esv.enter_context(tc.tile_pool(name="wvp", bufs=1))
            vinp = esv.enter_context(tc.tile_pool(name="vinp", bufs=2))
            vprj = esv.enter_context(tc.tile_pool(name="vprj", bufs=2, space="PSUM"))

            # mask tiles are allocated here but DMA'd after wv/vin0 (the V
            # pipeline start must not queue behind 4MB of mask traffic)
            mask_t = [maskp.tile([P, SQ], BF16, tag=f"m{kt}", name=f"mask{kt}")
                      for kt in range(NKT)]

            ctx_ps_cur = [None]      # ctx psum pair for the (j, qb) in flight

            def attn_open(j, qb):
                ctx_ps_cur[0] = [psc.tile([DK + 1, 512], F32, tag="ctx_ps",
                                          name=f"ctxps{j}_{qb}_{h2}", bufs=2)
                                 for h2 in range(2)]

            def attn_steps(pool, tag, sbufs, ptag, pbufs, j, qb, kts):
                """One scores-psum tile covering `kts` (1 or 2 k-tiles), one
                exp over the whole tile, then per-kt mask + ctx matmuls."""
                ctx_ps = ctx_ps_cur[0]
                W = SQ * len(kts)
                ps_s = pool.tile([P, W], F32, tag=tag,
                                 name=f"{tag}_{j}_{qb}_{kts[0]}", bufs=sbufs)
                for i, kt in enumerate(kts):
                    for h2 in range(2):
                        nc.tensor.matmul(
                            ps_s[:, i * SQ + h2 * 512:i * SQ + (h2 + 1) * 512],
                            lhsT=knt[j][h2 * DK:(h2 + 1) * DK,
                                        kt * P:(kt + 1) * P],
                            rhs=qn[j][h2 * DK:(h2 + 1) * DK,
                                      qb * 512:(qb + 1) * 512],
                            start=True, stop=True)
                p_t = ppool.tile([P, W], BF16, tag=ptag,
                                 name=f"p{ptag}_{j}_{qb}_{kts[0]}", bufs=pbufs)
                nc.scalar.activation(p_t[:, :], ps_s[:, :], AF.Exp)
                for i, kt in enumerate(kts):
                    nc.vector.tensor_tensor(
                        p_t[:, i * SQ:(i + 1) * SQ]
                            .rearrange("p (a b) -> p a b", a=2),
                        p_t[:, i * SQ:(i + 1) * SQ]
                            .rearrange("p (a b) -> p a b", a=2),
                        mask_t[kt][:, None, qb * 512:(qb + 1) * 512]
                            .to_broadcast((P, 2, 512)),
                        op=OP.mult)
                for i, kt in enumerate(kts):
                    for h2 in range(2):
                        h = 2 * j + h2
                        nc.tensor.matmul(
                            ctx_ps[h2][:, :],
                            lhsT=vau[h // 8][kt][:, h % 8, :],
                            rhs=p_t[:, i * SQ + h2 * 512:i * SQ + (h2 + 1) * 512],
                            start=(kt == 0), stop=(kt == NKT - 1))

            def attn_step(j, qb, kt):
                attn_steps(pss, "ps_s", 2, "p", 4, j, qb, [kt])

            def attn_close(j, qb):
                ctx_ps = ctx_ps_cur[0]
                if qb == 0:
                    ctx_j = qcp.tile([P, SQ], BF16, tag="qc", name=f"ctx{j}")
                    ctx.append(ctx_j)
                ctx_j = ctx[j]
                for h2 in range(2):
                    h = 2 * j + h2
                    nc.vector.tensor_scalar(
                        ctx_j[h2 * DK:(h2 + 1) * DK, qb * 512:(qb + 1) * 512],
                        ctx_ps[h2][0:DK, :], 1.0, None, op0=OP.mult)
                    # softmax sums rode the ctx matmul (ones column); stage to
                    # SBUF (DMA can't read PSUM), reshape through DRAM so the
                    # reciprocal runs 128 lanes wide (a [1,512] reciprocal
                    # monopolizes one DVE lane for ~3.4us and stalls the pipe)
                    sstage = ppool.tile([1, 512], F32, tag="sstage",
                                        name=f"sst{j}_{qb}_{h2}", bufs=2)
                    nc.vector.tensor_scalar(sstage[0:1, :],
                                            ctx_ps[h2][DK:DK + 1, :],
                                            1.0, None, op0=OP.mult)
                    nc.sync.dma_start(
                        out=sums_d[h:h + 1, qb * 512:(qb + 1) * 512],
                        in_=sstage[0:1, :])
                    # invert this qb's sums right away (128-wide via DRAM
                    # reshape) so the qb=1 close only assembles + multiplies
                    srow = ppool.tile([P, 4], F32, tag="srow",
                                      name=f"srow{j}_{qb}_{h2}", bufs=2)
                    nc.sync.dma_start(
                        out=srow[:, :],
                        in_=sums_d[h, qb * 512:(qb + 1) * 512]
                            .rearrange("(p c) -> p c", p=P))
                    nc.vector.reciprocal(srow[:, :], srow[:, :])
                    nc.sync.dma_start(
                        out=recip_d[h, qb * 512:(qb + 1) * 512]
                            .rearrange("(p c) -> p c", p=P),
                        in_=srow[:, :])
                if qb == 1:
                    bc = ppool.tile([P, SQ], F32, tag="bc", name=f"bc{j}", bufs=1)
                    nc.sync.dma_start(
                        out=bc[0:DK, :],
                        in_=recip_d[2 * j:2 * j + 1, :].to_broadcast((DK, SQ)))
                    nc.sync.dma_start(
                        out=bc[DK:P, :],
                        in_=recip_d[2 * j + 1:2 * j + 2, :].to_broadcast((DK, SQ)))
                    nc.vector.scalar_tensor_tensor(
                        ctx[j][:, :], ctx[j][:, :], 1.0, bc[:, :],
                        op0=OP.mult, op1=OP.mult)

            # V weights
            wv_t = []
            for ib in range(8):
                w_t = wvp.tile([P, D], BF16, tag=f"wv{ib}", name=f"wv{ib}")
                nc.sync.dma_start(out=w_t[:, :], in_=wvt[ib * P:(ib + 1) * P, :])
                wv_t.append(w_t)

            vin_cur = [None]

            def v_dma_chunk(sbi, tagpfx):
                vin_t = []
                for ib in range(8):
                    t = vinp.tile([P, 512], BF16, tag=f"in{ib}",
                                  name=f"vin{tagpfx}{ib}_{sbi}")
                    nc.sync.dma_start(
                        out=t[:, :],
                        in_=vt_in[ib * P:(ib + 1) * P, sbi * 512:(sbi + 1) * 512])
                    vin_t.append(t)
                vin_cur[0] = vin_t

            def v_group(st, ob):
                vin_t = vin_cur[0]
                str_ = st % 4
                pv = vprj.tile([P, 512], F32, tag="vprj", name=f"pv{st}_{ob}")
                for ib in range(8):
                    nc.tensor.matmul(
                        pv[:, :],
                        lhsT=vin_t[ib][:, str_ * P:(str_ + 1) * P],
                        rhs=wv_t[ib][:, ob * 512:(ob + 1) * 512],
                        start=(ib == 0), stop=False)
                nc.tensor.matmul(
                    pv[:, :], lhsT=onesr_sb[:, :],
                    rhs=bvr_sb[:, ob * 512:(ob + 1) * 512],
                    start=False, stop=True)
                nc.vector.tensor_scalar(
                    vau[ob][st][:, :, 0:DK],
                    pv[:, :].rearrange("p (a b) -> p a b", a=8),
                    1.0, None, op0=OP.mult)

            # V pass A (head-half ob0, feeds j=0..3) paced 1:1 with attention
            # steps of (j=0, qb=0)
            attn_open(0, 0)
            for st in range(NKT):
                if st % 4 == 0:
                    v_dma_chunk(st // 4, "A")
                    if st == 0:
                        for kt in range(NKT):
                            nc.sync.dma_start(out=mask_t[kt][:, :],
                                              in_=maskt[kt * P:(kt + 1) * P, :])
                v_group(st, 0)
                attn_step(0, 0, st)
            attn_close(0, 0)

            # out-proj weight DMAs issue here, overlapping remaining attention
            wo_t = []
            for ib in range(8):
                w_t = wop.tile([P, D], BF16, tag=f"wo{ib}", name=f"wo{ib}")
                nc.sync.dma_start(out=w_t[:, :], in_=wot[ib * P:(ib + 1) * P, :])
                wo_t.append(w_t)
            rmsw_sb = wrk.tile([P, D], BF16, name="rmsw_sb")
            nc.sync.dma_start(out=rmsw_sb[:, :], in_=rmsw[:, :])
            eps_t = wrk.tile([P, 1], F32, name="eps_t")
            nc.vector.memset(eps_t[:, :], EPS)

            # Attention j=0 (qb=1) then j=1..3: Act-saturated; V pass B (ob1,
            # feeds j=4..7) rides in the PE slack, one group every ~7 steps.
            vb_jobs = list(range(NKT))   # pass-B st groups still to emit
            groups_a = [(0, 1)] + [(j, qb) for j in range(1, 4) for qb in range(2)]
            nsteps = len(groups_a) * NKT
            placed = 0
            step_i = 0
            for (j, qb) in groups_a:
                attn_open(j, qb)
                for kt in range(NKT):
                    want = ((step_i + 1) * NKT) // nsteps
                    while placed < want:
                        st = vb_jobs[placed]
                        if st % 4 == 0:
                            v_dma_chunk(st // 4, "B")
                        v_group(st, 1)
                        placed += 1
                    attn_step(j, qb, kt)
                    step_i += 1
                attn_close(j, qb)
            while placed < NKT:
                st = vb_jobs[placed]
                if st % 4 == 0:
                    v_dma_chunk(st // 4, "B")
                v_group(st, 1)
                placed += 1

            esv.close()   # free wv / vin / V psum

            # attention j=4..7 (pure, Act-saturated)
            for j in range(4, NJ):
                for qb in range(2):
                    attn_open(j, qb)
                    for kt in range(NKT):
                        attn_step(j, qb, kt)
                    attn_close(j, qb)
            esp1.close()

            # kn / v_aug / mask / p no longer needed
            esa.close()
            es2.close()

            # ------------- out-proj + RMSNorm --------------------------
            with tc.tile_pool(name="outp", bufs=2) as outp, \
                 tc.tile_pool(name="scrp", bufs=2) as scrp, \
                 tc.tile_pool(name="pop", bufs=6, space="PSUM") as pop:
                # Software-pipelined st-loop: the db<7 accumulation matmuls of
                # a group have no dependency on ctx[7], so they run during the
                # j=7 normalize gate; the db=7 + bias matmuls trail two groups
                # behind.
                live = {}

                def out_partial(st):
                    o_sb = outp.tile([P, D], BF16, tag="o", name=f"o{st}",
                                     bufs=3)
                    pos = []
                    for ob in range(2):
                        po = pop.tile([P, 512], F32, tag="po", name=f"po{st}_{ob}")
                        for db in range(7):
                            nc.tensor.matmul(
                                po[:, :],
                                lhsT=ctx[db][:, st * P:(st + 1) * P],
                                rhs=wo_t[db][:, ob * 512:(ob + 1) * 512],
                                start=(db == 0), stop=False)
                        pos.append(po)
                    live[st] = (o_sb, pos)

                def out_finish(st):
                    o_sb, pos = live.pop(st)
                    for ob in range(2):
                        po = pos[ob]
                        nc.tensor.matmul(
                            po[:, :],
                            lhsT=ctx[7][:, st * P:(st + 1) * P],
                            rhs=wo_t[7][:, ob * 512:(ob + 1) * 512],
                            start=False, stop=False)
                        nc.tensor.matmul(
                            po[:, :], lhsT=onesr_sb[:, :],
                            rhs=bor_sb[:, ob * 512:(ob + 1) * 512],
                            start=False, stop=True)
                        # Act does the PSUM evacuation: the DVE is the tail
                        # bottleneck, Act is idle here
                        nc.scalar.copy(o_sb[:, ob * 512:(ob + 1) * 512],
                                       po[:, :])
                    sq_t = scrp.tile([P, D], BF16, tag="sq", name=f"sq{st}")
                    ssq = scrp.tile([P, 1], F32, tag="ssq", name=f"ssq{st}")
                    nc.vector.scalar_tensor_tensor(
                        sq_t[:, :], o_sb[:, :], 1.0, o_sb[:, :],
                        op0=OP.mult, op1=OP.mult, accum_out=ssq[:, :])
                    rms1 = scrp.tile([P, 1], F32, tag="rms1", name=f"rms1{st}")
                    nc.scalar.activation(rms1[:, :], ssq[:, :], AF.Sqrt,
                                         bias=eps_t[:, :], scale=1.0 / D)
                    nc.vector.reciprocal(rms1[:, :], rms1[:, :])
                    o_f = outp.tile([P, D], F32, tag="of", name=f"of{st}")
                    nc.vector.scalar_tensor_tensor(
                        o_f[:, :], o_sb[:, :], rms1[:, :], rmsw_sb[:, :],
                        op0=OP.mult, op1=OP.mult)
                    nc.sync.dma_start(out=out[st * P:(st + 1) * P, :],
                                      in_=o_f[:, :])

                for st in range(8):
                    out_partial(st)
                    if st >= 2:
                        out_finish(st - 2)
                out_finish(6)
                out_finish(7)

    nc.compile()
    return nc


_NC_CACHE = []


def _get_nc():
    if not _NC_CACHE:
        _NC_CACHE.append(build_nc())
    return _NC_CACHE[0]


def _fuse_na(w, b, na_w, na_b):
    """Fold the per-head NeuralAttention transform into the projection.

    reference: tanh(split(X @ w.T + b) @ na_w.T + na_b)
             = tanh(split(X @ (BD@w).T + (BD@b + tile(na_b))))
    with BD = blockdiag(na_w) over the H heads.  Returns (w_f.T, b_f).
    """
    w64 = w.astype(np.float64)
    wf = np.empty((D, D), np.float64)
    bf = np.empty((D,), np.float64)
    na64 = na_w.astype(np.float64)
    for h in range(H):
        sl = slice(h * DK, (h + 1) * DK)
        wf[sl, :] = na64 @ w64[sl, :]
        bf[sl] = na64 @ b.astype(np.float64)[sl] + na_b.astype(np.float64)
    return (np.ascontiguousarray(wf.T.astype(np.float32)),
            bf.astype(np.float32))


def _prep_in_maps(Q, K, V, mask, wq, bq, wk, bk, wv, bv, wo, bo,
                  na_q_w, na_q_b, na_k_w, na_k_b, temperature, rms_w):
    f = lambda x: np.asarray(x, dtype=np.float32)
    Q, K, V = f(Q), f(K), f(V)
    mask = np.asarray(mask)

    wqt_f, bq_f = _fuse_na(f(wq), f(bq), f(na_q_w), f(na_q_b))
    wkt_f, bk_f = _fuse_na(f(wk), f(bk), f(na_k_w), f(na_k_b))

    shared = dict(
        wqt=wqt_f.astype(ml_dtypes.bfloat16),
        wkt=wkt_f.astype(ml_dtypes.bfloat16),
        wvt=np.ascontiguousarray(f(wv).T).astype(ml_dtypes.bfloat16),
        wot=np.ascontiguousarray(f(wo).T).astype(ml_dtypes.bfloat16),
        bqt=np.ascontiguousarray(bq_f.reshape(8, P).T),
        bkt=np.ascontiguousarray(bk_f.reshape(8, P).T),
        bvr=_f32r_round(f(bv).reshape(1, D)),
        bor=_f32r_round(f(bo).reshape(1, D)),
        rmsw=np.ascontiguousarray(
            np.broadcast_to(f(rms_w), (P, D))).astype(ml_dtypes.bfloat16),
        onesr=np.ones((1, P), np.float32),
    )
    ts = 1.0 / (np.sqrt(DK).astype(np.float32) * f(temperature).reshape(H))
    tscp = np.empty((P, 8), np.float32)
    for j in range(NJ):
        tscp[0:DK, j] = ts[2 * j]
        tscp[DK:P, j] = ts[2 * j + 1]
    shared["tscp"] = tscp

    kts, vts = {}, {}
    for b in range(B):
        kts[b] = np.ascontiguousarray(K[b].T).astype(ml_dtypes.bfloat16)
        vts[b] = np.ascontiguousarray(V[b].T).astype(ml_dtypes.bfloat16)

    in_maps = []
    for c in range(NCORES):
        b, hf = divmod(c, 2)
        qsl = slice(hf * SQ, (hf + 1) * SQ)
        m = dict(shared)
        m["qt_in"] = np.ascontiguousarray(Q[b, qsl, :].T).astype(ml_dtypes.bfloat16)
        m["kt_in"] = kts[b]
        m["vt_in"] = vts[b]
        m["maskt"] = np.ascontiguousarray(
            mask[b, 0, qsl, :].T).astype(ml_dtypes.bfloat16)
        in_maps.append(m)
    return in_maps


def _run(in_maps, **kwargs):
    nc = _get_nc()
    return run_bass_kernel_spmd(nc, in_maps, core_ids=list(range(NCORES)), **kwargs)


def kernel(**inputs):
    in_maps = _prep_in_maps(**inputs)
    res = _run(in_maps)
    out = np.empty((B, S, D), np.float32)
    for c in range(NCORES):
        b, hf = divmod(c, 2)
        out[b, hf * SQ:(hf + 1) * SQ, :] = res.results[c]["out"]
    return out


# revision 53
# speedup vs baseline: 1.0200x; 1.0200x over previous
"""AdvancedMuonAttention Trainium2 kernel (8 NeuronCores, SPMD, no collectives).

Sharding: core c -> (batch b = c//2, query half q = c%2).  Each core computes
its [1024, 1024] slice of the output (including RMSNorm) entirely locally:
q-projection on its 1024 query rows, k/v-projections on the full 2048 keys of
its batch (duplicated across the 2 cores sharing a batch), attention, output
projection, RMSNorm.  The host shards inputs / reassembles outputs.

Device-side layout choices (validated by probes):
  - activations channels-first [D, S]; weights pre-transposed [D_in, D_out]
  - the per-head NeuralAttention transform is folded into wq/wk on the host
    (W' = blockdiag(na_w) @ W, b' = blockdiag(na_w) @ b + tile(na_b)), so
    the q/k projections emit tanh(...) directly from the projection PSUM
  - fp32r (fp32 rounded to 11 mantissa bits, full PE speed) for projections
  - bf16 for qn/kn/P/mask/v (2x DVE modes); fp32 PSUM accumulation
  - scoresT [k, q] orientation: softmax sums ride the ctx matmul via a ones
    column appended to v (M=65); division by sums is applied to ctx
  - exp without max subtraction (scores are bounded); masking = multiply
    exp(scores) by {0,1} mask

Schedule (v2): the Act engine's 256 exp instructions (~294us at 100% duty)
are the kernel floor, so the program is ordered to saturate Act as early as
possible and keep it saturated: K proj -> Q proj (tanh warms the exp table
set) -> V proj interleaved per-k-tile with attention (j=0, qb=0) -> rest of
attention with wo/rmsw prefetched underneath -> output projection + RMSNorm.
"""
import sys
import numpy as np
import ml_dtypes

sys.path.insert(0, "/opt/trn_rl_repo")

import concourse.bacc as bacc
import concourse.mybir as mybir
import concourse.tile as tile
from concourse.bass_utils import run_bass_kernel_spmd

F32 = mybir.dt.float32
F32R = mybir.dt.float32r
BF16 = mybir.dt.bfloat16

B, S, D, H, DK = 4, 2048, 1024, 16, 64
SQ = 1024            # query rows per core
P = 128              # partitions
NCORES = 8
NKT = S // P         # 16 k-tiles
NJ = H // 2          # 8 head pairs / d-block pairs
EPS = 1e-8


def _f32r_round(x):
    """RNE-round fp32 to 11 mantissa bits (the PE's fp32r operand format)."""
    u = np.ascontiguousarray(x, dtype=np.float32).view(np.uint32)
    r = ((u.astype(np.uint64) + 0x7FF + ((u >> 12) & 1)) & 0xFFFFF000).astype(np.uint32)
    return r.view(np.float32)


def build_nc():
    nc = bacc.Bacc("TRN2", target_bir_lowering=False)

    # inputs ----------------------------------------------------------------
    qt_in = nc.declare_dram_parameter("qt_in", [D, SQ], BF16, isOutput=False)
    kt_in = nc.declare_dram_parameter("kt_in", [D, S], BF16, isOutput=False)
    vt_in = nc.declare_dram_parameter("vt_in", [D, S], BF16, isOutput=False)
    maskt = nc.declare_dram_parameter("maskt", [S, SQ], BF16, isOutput=False)
    wqt = nc.declare_dram_parameter("wqt", [D, D], BF16, isOutput=False)
    wkt = nc.declare_dram_parameter("wkt", [D, D], BF16, isOutput=False)
    wvt = nc.declare_dram_parameter("wvt", [D, D], BF16, isOutput=False)
    wot = nc.declare_dram_parameter("wot", [D, D], BF16, isOutput=False)
    bqt = nc.declare_dram_parameter("bqt", [P, 8], F32, isOutput=False)
    bkt = nc.declare_dram_parameter("bkt", [P, 8], F32, isOutput=False)
    bvr = nc.declare_dram_parameter("bvr", [1, D], F32R, isOutput=False)
    bor = nc.declare_dram_parameter("bor", [1, D], F32R, isOutput=False)
    tscp = nc.declare_dram_parameter("tscp", [P, 8], F32, isOutput=False)
    rmsw = nc.declare_dram_parameter("rmsw", [P, D], BF16, isOutput=False)
    onesr = nc.declare_dram_parameter("onesr", [1, P], F32R, isOutput=False)
    out = nc.declare_dram_parameter("out", [SQ, D], F32, isOutput=True)

    sums_d = nc.dram_tensor("sums_d", [H, SQ], F32)
    recip_d = nc.dram_tensor("recip_d", [H, SQ], F32)

    AF = mybir.ActivationFunctionType
    OP = mybir.AluOpType

    with tile.TileContext(nc) as tc:
        import contextlib
        es = contextlib.ExitStack()
        with es:
            # long-lived pools
            const = es.enter_context(tc.tile_pool(name="const", bufs=1))
            qcp = es.enter_context(tc.tile_pool(name="qcp", bufs=9))
            wrk = es.enter_context(tc.tile_pool(name="wrk", bufs=1))
            wop = es.enter_context(tc.tile_pool(name="wop", bufs=1))
            es2 = es.enter_context(contextlib.ExitStack())
            knp = es2.enter_context(tc.tile_pool(name="knp", bufs=1))
            vap = es2.enter_context(tc.tile_pool(name="vap", bufs=1))

            # constants
            onesr_sb = const.tile([1, P], F32R, name="onesr_sb")
            nc.sync.dma_start(out=onesr_sb[:, :], in_=onesr[:, :])
            bvr_sb = const.tile([1, D], F32R, name="bvr_sb")
            nc.sync.dma_start(out=bvr_sb[:, :], in_=bvr[:, :])
            bor_sb = const.tile([1, D], F32R, name="bor_sb")
            nc.sync.dma_start(out=bor_sb[:, :], in_=bor[:, :])
            bqt_sb = const.tile([P, 8], F32, name="bqt_sb")
            nc.sync.dma_start(out=bqt_sb[:, :], in_=bqt[:, :])
            bkt_sb = const.tile([P, 8], F32, name="bkt_sb")
            nc.sync.dma_start(out=bkt_sb[:, :], in_=bkt[:, :])
            tscp_sb = const.tile([P, 8], F32, name="tscp_sb")
            nc.sync.dma_start(out=tscp_sb[:, :], in_=tscp[:, :])

            # long-lived tensors.  v is split by head-half (ob): heads 0-7
            # feed attention j=0..3, heads 8-15 feed j=4..7 — this lets the
            # ob1 half of the V projection run underneath Act-saturated
            # attention instead of blocking it.
            knt = [knp.tile([P, S], BF16, tag=f"kn{j}", name=f"knt{j}")
                   for j in range(NJ)]
            vau = [[vap.tile([P, 8, DK + 1], BF16, tag=f"v{ob}_{st}",
                             name=f"vaug{ob}_{st}") for st in range(NKT)]
                   for ob in range(2)]
            qn = [qcp.tile([P, SQ], BF16, tag="qc", name=f"qn{j}")
                  for j in range(NJ)]
            for ob in range(2):
                for st in range(NKT):
                    nc.vector.memset(vau[ob][st][:, :, DK:DK + 1], 1.0)

            # transient input pool for K/Q chunks (freed before attention)
            eskq = contextlib.ExitStack()
            inp = eskq.enter_context(tc.tile_pool(name="inp", bufs=2))
            wqp = eskq.enter_context(tc.tile_pool(name="wqp", bufs=1))
            wq_t = []

            # ---------------- phase K: kn = tanh(K @ (naK@wk).T + b') -------
            with tc.tile_pool(name="wkp", bufs=1) as wkp, \
                 tc.tile_pool(name="kqprj", bufs=4, space="PSUM") as kqprj:
                wk_t = []
                for ib in range(8):
                    w_t = wkp.tile([P, D], BF16, tag=f"wk{ib}", name=f"wk{ib}")
                    nc.sync.dma_start(out=w_t[:, :], in_=wkt[ib * P:(ib + 1) * P, :])
                    wk_t.append(w_t)
                for sbi in range(4):
                    kin_t = []
                    for ib in range(8):
                        t = inp.tile([P, 512], BF16, tag=f"in{ib}", name=f"kin{ib}_{sbi}")
                        nc.sync.dma_start(
                            out=t[:, :],
                            in_=kt_in[ib * P:(ib + 1) * P, sbi * 512:(sbi + 1) * 512])
                        kin_t.append(t)
                    if sbi == 0:
                        # prefetch the Q weights under the K projection
                        for ib in range(8):
                            w_t = wqp.tile([P, D], BF16, tag=f"wq{ib}",
                                           name=f"wq{ib}")
                            nc.sync.dma_start(
                                out=w_t[:, :],
                                in_=wqt[ib * P:(ib + 1) * P, :])
                            wq_t.append(w_t)
                    for j in range(NJ):
                        pk = kqprj.tile([P, 512], F32, tag="prj", name=f"pk{sbi}_{j}")
                        for ib in range(8):
                            nc.tensor.matmul(
                                pk[:, :],
                                lhsT=wk_t[ib][:, j * P:(j + 1) * P],
                                rhs=kin_t[ib][:, :],
                                start=(ib == 0), stop=(ib == 7))
                        nc.scalar.activation(
                            knt[j][:, sbi * 512:(sbi + 1) * 512], pk[:, :],
                            AF.Tanh, bias=bkt_sb[:, j:j + 1])

            # ---------------- phase Q ---------------------------------------
            with tc.tile_pool(name="qprj", bufs=4, space="PSUM") as kqprj:
                if True:
                    for sbi in range(2):
                        qin_t = []
                        for ib in range(8):
                            t = inp.tile([P, 512], BF16, tag=f"in{ib}", name=f"qin{ib}_{sbi}")
                            nc.sync.dma_start(
                                out=t[:, :],
                                in_=qt_in[ib * P:(ib + 1) * P, sbi * 512:(sbi + 1) * 512])
                            qin_t.append(t)
                        for j in range(NJ):
                            pq = kqprj.tile([P, 512], F32, tag="prj", name=f"pq{sbi}_{j}")
                            for ib in range(8):
                                nc.tensor.matmul(
                                    pq[:, :],
                                    lhsT=wq_t[ib][:, j * P:(j + 1) * P],
                                    rhs=qin_t[ib][:, :],
                                    start=(ib == 0), stop=(ib == 7))
                            nc.scalar.activation(
                                qn[j][:, sbi * 512:(sbi + 1) * 512], pq[:, :],
                                AF.Tanh, bias=bqt_sb[:, j:j + 1])
                    # fold 1/(sqrt(DK)*temp_h) into qn
                    for j in range(NJ):
                        nc.vector.tensor_scalar_mul(qn[j][:, :], qn[j][:, :],
                                                    tscp_sb[:, j:j + 1])

            eskq.close()   # free the K/Q input pool

            # ---------------- V proj + attention (overlapped) ---------------
            ctx = []
            esa = contextlib.ExitStack()
            maskp = esa.enter_context(tc.tile_pool(name="maskp", bufs=1))
            ppool = esa.enter_context(tc.tile_pool(name="pp", bufs=4))
            psc = esa.enter_context(tc.tile_pool(name="psc", bufs=2, space="PSUM"))

            esp1 = contextlib.ExitStack()
            pss = esp1.enter_context(tc.tile_pool(name="pss1", bufs=1,
                                                  space="PSUM"))

            esv = contextlib.ExitStack()
            wvp = esv.enter_context(tc.tile_pool(name="wvp", bufs=1))
            vinp = esv.enter_context(tc.tile_pool(name="vinp", bufs=2))
            vprj = esv.enter_context(tc.tile_pool(name="vprj", bufs=2, space="PSUM"))

            # mask tiles are allocated here but DMA'd after wv/vin0 (the V
            # pipeline start must not queue behind 4MB of mask traffic)
            mask_t = [maskp.tile([P, SQ], BF16, tag=f"m{kt}", name=f"mask{kt}")
                      for kt in range(NKT)]

            ctx_ps_cur = [None]      # ctx psum pair for the (j, qb) in flight

            def attn_open(j, qb):
                ctx_ps_cur[0] = [psc.tile([DK + 1, 512], F32, tag="ctx_ps",
                                          name=f"ctxps{j}_{qb}_{h2}", bufs=2)
                                 for h2 in range(2)]

            def attn_steps(pool, tag, sbufs, ptag, pbufs, j, qb, kts):
                """One scores-psum tile covering `kts` (1 or 2 k-tiles), one
                exp over the whole tile, then per-kt mask + ctx matmuls."""
                ctx_ps = ctx_ps_cur[0]
                W = SQ * len(kts)
                ps_s = pool.tile([P, W], F32, tag=tag,
                                 name=f"{tag}_{j}_{qb}_{kts[0]}", bufs=sbufs)
                for i, kt in enumerate(kts):
                    for h2 in range(2):
                        nc.tensor.matmul(
                            ps_s[:, i * SQ + h2 * 512:i * SQ + (h2 + 1) * 512],
                            lhsT=knt[j][h2 * DK:(h2 + 1) * DK,
                                        kt * P:(kt + 1) * P],
                            rhs=qn[j][h2 * DK:(h2 + 1) * DK,
                                      qb * 512:(qb + 1) * 512],
                            start=True, stop=True)
                p_t = ppool.tile([P, W], BF16, tag=ptag,
                                 name=f"p{ptag}_{j}_{qb}_{kts[0]}", bufs=pbufs)
                nc.scalar.activation(p_t[:, :], ps_s[:, :], AF.Exp)
                for i, kt in enumerate(kts):
                    nc.vector.tensor_tensor(
                        p_t[:, i * SQ:(i + 1) * SQ]
                            .rearrange("p (a b) -> p a b", a=2),
                        p_t[:, i * SQ:(i + 1) * SQ]
                            .rearrange("p (a b) -> p a b", a=2),
                        mask_t[kt][:, None, qb * 512:(qb + 1) * 512]
                            .to_broadcast((P, 2, 512)),
                        op=OP.mult)
                for i, kt in enumerate(kts):
                    for h2 in range(2):
                        h = 2 * j + h2
                        nc.tensor.matmul(
                            ctx_ps[h2][:, :],
                            lhsT=vau[h // 8][kt][:, h % 8, :],
                            rhs=p_t[:, i * SQ + h2 * 512:i * SQ + (h2 + 1) * 512],
                            start=(kt == 0), stop=(kt == NKT - 1))

            def attn_step(j, qb, kt):
                attn_steps(pss, "ps_s", 2, "p", 4, j, qb, [kt])

            def attn_close(j, qb):
                ctx_ps = ctx_ps_cur[0]
                if qb == 0:
                    ctx_j = qcp.tile([P, SQ], BF16, tag="qc", name=f"ctx{j}")
                    ctx.append(ctx_j)
                ctx_j = ctx[j]
                for h2 in range(2):
                    h = 2 * j + h2
                    nc.vector.tensor_scalar(
                        ctx_j[h2 * DK:(h2 + 1) * DK, qb * 512:(qb + 1) * 512],
                        ctx_ps[h2][0:DK, :], 1.0, None, op0=OP.mult)
                    # softmax sums rode the ctx matmul (ones column); stage to
                    # SBUF (DMA can't read PSUM), reshape through DRAM so the
                    # reciprocal runs 128 lanes wide (a [1,512] reciprocal
                    # monopolizes one DVE lane for ~3.4us and stalls the pipe)
                    sstage = ppool.tile([1, 512], F32, tag="sstage",
                                        name=f"sst{j}_{qb}_{h2}", bufs=2)
                    nc.vector.tensor_scalar(sstage[0:1, :],
                                            ctx_ps[h2][DK:DK + 1, :],
                                            1.0, None, op0=OP.mult)
                    nc.sync.dma_start(
                        out=sums_d[h:h + 1, qb * 512:(qb + 1) * 512],
                        in_=sstage[0:1, :])
                    # invert this qb's sums right away (128-wide via DRAM
                    # reshape) so the qb=1 close only assembles + multiplies
                    srow = ppool.tile([P, 4], F32, tag="srow",
                                      name=f"srow{j}_{qb}_{h2}", bufs=2)
                    nc.sync.dma_start(
                        out=srow[:, :],
                        in_=sums_d[h, qb * 512:(qb + 1) * 512]
                            .rearrange("(p c) -> p c", p=P))
                    nc.vector.reciprocal(srow[:, :], srow[:, :])
                    nc.sync.dma_start(
                        out=recip_d[h, qb * 512:(qb + 1) * 512]
                            .rearrange("(p c) -> p c", p=P),
                        in_=srow[:, :])
                if qb == 1:
                    bc = ppool.tile([P, SQ], F32, tag="bc", name=f"bc{j}", bufs=1)
                    nc.sync.dma_start(
                        out=bc[0:DK, :],
                        in_=recip_d[2 * j:2 * j + 1, :].to_broadcast((DK, SQ)))
                    nc.sync.dma_start(
                        out=bc[DK:P, :],
                        in_=recip_d[2 * j + 1:2 * j + 2, :].to_broadcast((DK, SQ)))
                    nc.vector.scalar_tensor_tensor(
                        ctx[j][:, :], ctx[j][:, :], 1.0, bc[:, :],
                        op0=OP.mult, op1=OP.mult)

            # V weights
            wv_t = []
            for ib in range(8):
                w_t = wvp.tile([P, D], BF16, tag=f"wv{ib}", name=f"wv{ib}")
                nc.sync.dma_start(out=w_t[:, :], in_=wvt[ib * P:(ib + 1) * P, :])
                wv_t.append(w_t)

            vin_cur = [None]

            def v_dma_chunk(sbi, tagpfx):
                vin_t = []
                for ib in range(8):
                    t = vinp.tile([P, 512], BF16, tag=f"in{ib}",
                                  name=f"vin{tagpfx}{ib}_{sbi}")
                    nc.sync.dma_start(
                        out=t[:, :],
                        in_=vt_in[ib * P:(ib + 1) * P, sbi * 512:(sbi + 1) * 512])
                    vin_t.append(t)
                vin_cur[0] = vin_t

            def v_group(st, ob):
                vin_t = vin_cur[0]
                str_ = st % 4
                pv = vprj.tile([P, 512], F32, tag="vprj", name=f"pv{st}_{ob}")
                for ib in range(8):
                    nc.tensor.matmul(
                        pv[:, :],
                        lhsT=vin_t[ib][:, str_ * P:(str_ + 1) * P],
                        rhs=wv_t[ib][:, ob * 512:(ob + 1) * 512],
                        start=(ib == 0), stop=False)
                nc.tensor.matmul(
                    pv[:, :], lhsT=onesr_sb[:, :],
                    rhs=bvr_sb[:, ob * 512:(ob + 1) * 512],
                    start=False, stop=True)
                nc.vector.tensor_scalar(
                    vau[ob][st][:, :, 0:DK],
                    pv[:, :].rearrange("p (a b) -> p a b", a=8),
                    1.0, None, op0=OP.mult)

            # V pass A (head-half ob0, feeds j=0..3) paced 1:1 with attention
            # steps of (j=0, qb=0)
            attn_open(0, 0)
            for st in range(NKT):
                if st % 4 == 0:
                    v_dma_chunk(st // 4, "A")
                    if st == 0:
                        for kt in range(NKT):
                            nc.sync.dma_start(out=mask_t[kt][:, :],
                                              in_=maskt[kt * P:(kt + 1) * P, :])
                v_group(st, 0)
                attn_step(0, 0, st)
            attn_close(0, 0)

            # out-proj weight DMAs issue here, overlapping remaining attention
            wo_t = []
            for ib in range(8):
                w_t = wop.tile([P, D], BF16, tag=f"wo{ib}", name=f"wo{ib}")
                nc.sync.dma_start(out=w_t[:, :], in_=wot[ib * P:(ib + 1) * P, :])
                wo_t.append(w_t)
            rmsw_sb = wrk.tile([P, D], BF16, name="rmsw_sb")
            nc.sync.dma_start(out=rmsw_sb[:, :], in_=rmsw[:, :])
            eps_t = wrk.tile([P, 1], F32, name="eps_t")
            nc.vector.memset(eps_t[:, :], EPS)

            # Attention j=0 (qb=1) then j=1..3: Act-saturated; V pass B (ob1,
            # feeds j=4..7) rides in the PE slack, one group every ~7 steps.
            vb_jobs = list(range(NKT))   # pass-B st groups still to emit
            groups_a = [(0, 1)] + [(j, qb) for j in range(1, 4) for qb in range(2)]
            nsteps = len(groups_a) * NKT
            placed = 0
            step_i = 0
            for (j, qb) in groups_a:
                attn_open(j, qb)
                for kt in range(NKT):
                    want = ((step_i + 1) * NKT) // nsteps
                    while placed < want:
                        st = vb_jobs[placed]
                        if st % 4 == 0:
                            v_dma_chunk(st // 4, "B")
                        v_group(st, 1)
                        placed += 1
                    attn_step(j, qb, kt)
                    step_i += 1
                attn_close(j, qb)
            while placed < NKT:
                st = vb_jobs[placed]
                if st % 4 == 0:
                    v_dma_chunk(st // 4, "B")
                v_group(st, 1)
                placed += 1

            esv.close()   # free wv / vin / V psum

            # attention j=4..7 (pure, Act-saturated)
            for j in range(4, NJ):
                for qb in range(2):
                    attn_open(j, qb)
                    for kt in range(NKT):
                        attn_step(j, qb, kt)
                    attn_close(j, qb)
            esp1.close()

            # kn / v_aug / mask / p no longer needed
            esa.close()
            es2.close()

            # ------------- out-proj + RMSNorm --------------------------
            with tc.tile_pool(name="outp", bufs=2) as outp, \
                 tc.tile_pool(name="scrp", bufs=2) as scrp, \
                 tc.tile_pool(name="pop", bufs=6, space="PSUM") as pop:
                for st in range(8):
                    o_sb = outp.tile([P, D], BF16, tag="o", name=f"o{st}")
                    for ob in range(2):
                        po = pop.tile([P, 512], F32, tag="po", name=f"po{st}_{ob}")
                        for db in range(8):
                            nc.tensor.matmul(
                                po[:, :],
                                lhsT=ctx[db][:, st * P:(st + 1) * P],
                                rhs=wo_t[db][:, ob * 512:(ob + 1) * 512],
                                start=(db == 0), stop=False)
                        nc.tensor.matmul(
                            po[:, :], lhsT=onesr_sb[:, :],
                            rhs=bor_sb[:, ob * 512:(ob + 1) * 512],
                            start=False, stop=True)
                        # Act does the PSUM evacuation: the DVE is the tail
                        # bottleneck, Act is idle here
                        nc.scalar.copy(o_sb[:, ob * 512:(ob + 1) * 512],
                                       po[:, :])
                    sq_t = scrp.tile([P, D], BF16, tag="sq", name=f"sq{st}")
                    ssq = scrp.tile([P, 1], F32, tag="ssq", name=f"ssq{st}")
                    nc.vector.scalar_tensor_tensor(
                        sq_t[:, :], o_sb[:, :], 1.0, o_sb[:, :],
                        op0=OP.mult, op1=OP.mult, accum_out=ssq[:, :])
                    rms1 = scrp.tile([P, 1], F32, tag="rms1", name=f"rms1{st}")
                    nc.scalar.activation(rms1[:, :], ssq[:, :], AF.Sqrt,
                                         bias=eps_t[:, :], scale=1.0 / D)
                    nc.vector.reciprocal(rms1[:, :], rms1[:, :])
                    o_f = outp.tile([P, D], F32, tag="of", name=f"of{st}")
                    nc.vector.scalar_tensor_tensor(
                        o_f[:, :], o_sb[:, :], rms1[:, :], rmsw_sb[:, :],
                        op0=OP.mult, op1=OP.mult)
                    nc.sync.dma_start(out=out[st * P:(st + 1) * P, :],
                                      in_=o_f[:, :])

    nc.compile()
    return nc


_NC_CACHE = []


def _get_nc():
    if not _NC_CACHE:
        _NC_CACHE.append(build_nc())
    return _NC_CACHE[0]


def _fuse_na(w, b, na_w, na_b):
    """Fold the per-head NeuralAttention transform into the projection.

    reference: tanh(split(X @ w.T + b) @ na_w.T + na_b)
             = tanh(split(X @ (BD@w).T + (BD@b + tile(na_b))))
    with BD = blockdiag(na_w) over the H heads.  Returns (w_f.T, b_f).
    """
    w64 = w.astype(np.float64)
    wf = np.empty((D, D), np.float64)
    bf = np.empty((D,), np.float64)
    na64 = na_w.astype(np.float64)
    for h in range(H):
        sl = slice(h * DK, (h + 1) * DK)
        wf[sl, :] = na64 @ w64[sl, :]
        bf[sl] = na64 @ b.astype(np.float64)[sl] + na_b.astype(np.float64)
    return (np.ascontiguousarray(wf.T.astype(np.float32)),
            bf.astype(np.float32))


def _prep_in_maps(Q, K, V, mask, wq, bq, wk, bk, wv, bv, wo, bo,
                  na_q_w, na_q_b, na_k_w, na_k_b, temperature, rms_w):
    f = lambda x: np.asarray(x, dtype=np.float32)
    Q, K, V = f(Q), f(K), f(V)
    mask = np.asarray(mask)

    wqt_f, bq_f = _fuse_na(f(wq), f(bq), f(na_q_w), f(na_q_b))
    wkt_f, bk_f = _fuse_na(f(wk), f(bk), f(na_k_w), f(na_k_b))

    shared = dict(
        wqt=wqt_f.astype(ml_dtypes.bfloat16),
        wkt=wkt_f.astype(ml_dtypes.bfloat16),
        wvt=np.ascontiguousarray(f(wv).T).astype(ml_dtypes.bfloat16),
        wot=np.ascontiguousarray(f(wo).T).astype(ml_dtypes.bfloat16),
        bqt=np.ascontiguousarray(bq_f.reshape(8, P).T),
        bkt=np.ascontiguousarray(bk_f.reshape(8, P).T),
        bvr=_f32r_round(f(bv).reshape(1, D)),
        bor=_f32r_round(f(bo).reshape(1, D)),
        rmsw=np.ascontiguousarray(
            np.broadcast_to(f(rms_w), (P, D))).astype(ml_dtypes.bfloat16),
        onesr=np.ones((1, P), np.float32),
    )
    ts = 1.0 / (np.sqrt(DK).astype(np.float32) * f(temperature).reshape(H))
    tscp = np.empty((P, 8), np.float32)
    for j in range(NJ):
        tscp[0:DK, j] = ts[2 * j]
        tscp[DK:P, j] = ts[2 * j + 1]
    shared["tscp"] = tscp

    kts, vts = {}, {}
    for b in range(B):
        kts[b] = np.ascontiguousarray(K[b].T).astype(ml_dtypes.bfloat16)
        vts[b] = np.ascontiguousarray(V[b].T).astype(ml_dtypes.bfloat16)

    in_maps = []
    for c in range(NCORES):
        b, hf = divmod(c, 2)
        qsl = slice(hf * SQ, (hf + 1) * SQ)
        m = dict(shared)
        m["qt_in"] = np.ascontiguousarray(Q[b, qsl, :].T).astype(ml_dtypes.bfloat16)
        m["kt_in"] = kts[b]
        m["vt_in"] = vts[b]
        m["maskt"] = np.ascontiguousarray(
            mask[b, 0, qsl, :].T).astype(ml_dtypes.bfloat16)
        in_maps.append(m)
    return in_maps


def _run(in_maps, **kwargs):
    nc = _get_nc()
    return run_bass_kernel_spmd(nc, in_maps, core_ids=list(range(NCORES)), **kwargs)


def kernel(**inputs):
    in_maps = _prep_in_maps(**inputs)
    res = _run(in_maps)
    out = np.empty((B, S, D), np.float32)
    for c in range(NCORES):
        b, hf = divmod(c, 2)
        out[b, hf * SQ:(hf + 1) * SQ, :] = res.results[c]["out"]
    return out


# revision 63
# speedup vs baseline: 1.0217x; 1.0017x over previous
"""AdvancedMuonAttention Trainium2 kernel (8 NeuronCores, SPMD, no collectives).

Sharding: core c -> (batch b = c//2, query half q = c%2).  Each core computes
its [1024, 1024] slice of the output (including RMSNorm) entirely locally:
q-projection on its 1024 query rows, k/v-projections on the full 2048 keys of
its batch (duplicated across the 2 cores sharing a batch), attention, output
projection, RMSNorm.  The host shards inputs / reassembles outputs.

Device-side layout choices (validated by probes):
  - activations channels-first [D, S]; weights pre-transposed [D_in, D_out]
  - the per-head NeuralAttention transform is folded into wq/wk on the host
    (W' = blockdiag(na_w) @ W, b' = blockdiag(na_w) @ b + tile(na_b)), so
    the q/k projections emit tanh(...) directly from the projection PSUM
  - fp32r (fp32 rounded to 11 mantissa bits, full PE speed) for projections
  - bf16 for qn/kn/P/mask/v (2x DVE modes); fp32 PSUM accumulation
  - scoresT [k, q] orientation: softmax sums ride the ctx matmul via a ones
    column appended to v (M=65); division by sums is applied to ctx
  - exp without max subtraction (scores are bounded); masking = multiply
    exp(scores) by {0,1} mask

Schedule (v2): the Act engine's 256 exp instructions (~294us at 100% duty)
are the kernel floor, so the program is ordered to saturate Act as early as
possible and keep it saturated: K proj -> Q proj (tanh warms the exp table
set) -> V proj interleaved per-k-tile with attention (j=0, qb=0) -> rest of
attention with wo/rmsw prefetched underneath -> output projection + RMSNorm.
"""
import sys
import numpy as np
import ml_dtypes

sys.path.insert(0, "/opt/trn_rl_repo")

import concourse.bacc as bacc
import concourse.mybir as mybir
import concourse.tile as tile
from concourse.bass_utils import run_bass_kernel_spmd

F32 = mybir.dt.float32
F32R = mybir.dt.float32r
BF16 = mybir.dt.bfloat16

B, S, D, H, DK = 4, 2048, 1024, 16, 64
SQ = 1024            # query rows per core
P = 128              # partitions
NCORES = 8
NKT = S // P         # 16 k-tiles
NJ = H // 2          # 8 head pairs / d-block pairs
EPS = 1e-8


def _f32r_round(x):
    """RNE-round fp32 to 11 mantissa bits (the PE's fp32r operand format)."""
    u = np.ascontiguousarray(x, dtype=np.float32).view(np.uint32)
    r = ((u.astype(np.uint64) + 0x7FF + ((u >> 12) & 1)) & 0xFFFFF000).astype(np.uint32)
    return r.view(np.float32)


def build_nc():
    nc = bacc.Bacc("TRN2", target_bir_lowering=False)

    # inputs ----------------------------------------------------------------
    qt_in = nc.declare_dram_parameter("qt_in", [D, SQ], BF16, isOutput=False)
    kt_in = nc.declare_dram_parameter("kt_in", [D, S], BF16, isOutput=False)
    vt_in = nc.declare_dram_parameter("vt_in", [D, S], BF16, isOutput=False)
    maskt = nc.declare_dram_parameter("maskt", [S, SQ], BF16, isOutput=False)
    wqt = nc.declare_dram_parameter("wqt", [D, D], BF16, isOutput=False)
    wkt = nc.declare_dram_parameter("wkt", [D, D], BF16, isOutput=False)
    wvt = nc.declare_dram_parameter("wvt", [D, D], BF16, isOutput=False)
    wot = nc.declare_dram_parameter("wot", [D, D], BF16, isOutput=False)
    bqt = nc.declare_dram_parameter("bqt", [P, 8], F32, isOutput=False)
    bkt = nc.declare_dram_parameter("bkt", [P, 8], F32, isOutput=False)
    bvr = nc.declare_dram_parameter("bvr", [1, D], F32R, isOutput=False)
    bor = nc.declare_dram_parameter("bor", [1, D], F32R, isOutput=False)
    tscp = nc.declare_dram_parameter("tscp", [P, 8], F32, isOutput=False)
    rmsw = nc.declare_dram_parameter("rmsw", [P, D], BF16, isOutput=False)
    onesr = nc.declare_dram_parameter("onesr", [1, P], F32R, isOutput=False)
    out = nc.declare_dram_parameter("out", [SQ, D], F32, isOutput=True)

    sums_d = nc.dram_tensor("sums_d", [H, SQ], F32)
    recip_d = nc.dram_tensor("recip_d", [H, SQ], F32)

    AF = mybir.ActivationFunctionType
    OP = mybir.AluOpType

    with tile.TileContext(nc) as tc:
        import contextlib
        es = contextlib.ExitStack()
        with es:
            # long-lived pools
            const = es.enter_context(tc.tile_pool(name="const", bufs=1))
            qcp = es.enter_context(tc.tile_pool(name="qcp", bufs=9))
            wrk = es.enter_context(tc.tile_pool(name="wrk", bufs=1))
            wop = es.enter_context(tc.tile_pool(name="wop", bufs=1))
            es2 = es.enter_context(contextlib.ExitStack())
            knp = es2.enter_context(tc.tile_pool(name="knp", bufs=1))
            vap = es2.enter_context(tc.tile_pool(name="vap", bufs=1))

            # constant tiles (DMAs emitted after the K-critical-path DMAs)
            onesr_sb = const.tile([1, P], F32R, name="onesr_sb")
            bvr_sb = const.tile([1, D], F32R, name="bvr_sb")
            bor_sb = const.tile([1, D], F32R, name="bor_sb")
            bqt_sb = const.tile([P, 8], F32, name="bqt_sb")
            bkt_sb = const.tile([P, 8], F32, name="bkt_sb")
            tscp_sb = const.tile([P, 8], F32, name="tscp_sb")

            # long-lived tensors.  v is split by head-half (ob): heads 0-7
            # feed attention j=0..3, heads 8-15 feed j=4..7 — this lets the
            # ob1 half of the V projection run underneath Act-saturated
            # attention instead of blocking it.
            knt = [knp.tile([P, S], BF16, tag=f"kn{j}", name=f"knt{j}")
                   for j in range(NJ)]
            vau = [[vap.tile([P, 8, DK + 1], BF16, tag=f"v{ob}_{st}",
                             name=f"vaug{ob}_{st}") for st in range(NKT)]
                   for ob in range(2)]
            qn = [qcp.tile([P, SQ], BF16, tag="qc", name=f"qn{j}")
                  for j in range(NJ)]
            for ob in range(2):
                for st in range(NKT):
                    nc.vector.memset(vau[ob][st][:, :, DK:DK + 1], 1.0)

            # V-input pool opens early so vin chunk 0 can prefetch under K
            esvin = contextlib.ExitStack()
            vinp = esvin.enter_context(tc.tile_pool(name="vinp", bufs=2))
            wv_t = []
            vin_cur = [None]

            def v_dma_chunk(sbi, tagpfx):
                vin_t = []
                for ib in range(8):
                    t = vinp.tile([P, 512], BF16, tag=f"in{ib}",
                                  name=f"vin{tagpfx}{ib}_{sbi}")
                    nc.sync.dma_start(
                        out=t[:, :],
                        in_=vt_in[ib * P:(ib + 1) * P, sbi * 512:(sbi + 1) * 512])
                    vin_t.append(t)
                vin_cur[0] = vin_t

            # transient input pool for K/Q chunks (freed before attention)
            eskq = contextlib.ExitStack()
            inp = eskq.enter_context(tc.tile_pool(name="inp", bufs=2))
            wqp = eskq.enter_context(tc.tile_pool(name="wqp", bufs=1))
            wq_t = []

            # ---------------- phase K: kn = tanh(K @ (naK@wk).T + b') -------
            with tc.tile_pool(name="wkp", bufs=1) as wkp, \
                 tc.tile_pool(name="kqprj", bufs=4, space="PSUM") as kqprj:
                wk_t = []
                for ib in range(8):
                    w_t = wkp.tile([P, D], BF16, tag=f"wk{ib}", name=f"wk{ib}")
                    nc.sync.dma_start(out=w_t[:, :], in_=wkt[ib * P:(ib + 1) * P, :])
                    wk_t.append(w_t)
                for sbi in range(4):
                    kin_t = []
                    for ib in range(8):
                        t = inp.tile([P, 512], BF16, tag=f"in{ib}", name=f"kin{ib}_{sbi}")
                        nc.sync.dma_start(
                            out=t[:, :],
                            in_=kt_in[ib * P:(ib + 1) * P, sbi * 512:(sbi + 1) * 512])
                        kin_t.append(t)
                    if sbi == 0:
                        # prefetch the Q weights under the K projection
                        for ib in range(8):
                            w_t = wqp.tile([P, D], BF16, tag=f"wq{ib}",
                                           name=f"wq{ib}")
                            nc.sync.dma_start(
                                out=w_t[:, :],
                                in_=wqt[ib * P:(ib + 1) * P, :])
                            wq_t.append(w_t)
                        # consts (queued behind the K critical path)
                        nc.sync.dma_start(out=bkt_sb[:, :], in_=bkt[:, :])
                        nc.sync.dma_start(out=bqt_sb[:, :], in_=bqt[:, :])
                        nc.sync.dma_start(out=tscp_sb[:, :], in_=tscp[:, :])
                        nc.sync.dma_start(out=onesr_sb[:, :], in_=onesr[:, :])
                        nc.sync.dma_start(out=bvr_sb[:, :], in_=bvr[:, :])
                        nc.sync.dma_start(out=bor_sb[:, :], in_=bor[:, :])
                    if sbi == 1:
                        # prefetch V weights (into the wo slots — same shape
                        # and dtype; the later wo DMA then naturally waits
                        # until wv is dead) and the first V input chunk
                        for ib in range(8):
                            w_t = wop.tile([P, D], BF16, tag=f"wo{ib}",
                                           name=f"wv{ib}")
                            nc.sync.dma_start(
                                out=w_t[:, :],
                                in_=wvt[ib * P:(ib + 1) * P, :])
                            wv_t.append(w_t)
                        v_dma_chunk(0, "A")
                    for j in range(NJ):
                        pk = kqprj.tile([P, 512], F32, tag="prj", name=f"pk{sbi}_{j}")
                        for ib in range(8):
                            nc.tensor.matmul(
                                pk[:, :],
                                lhsT=wk_t[ib][:, j * P:(j + 1) * P],
                                rhs=kin_t[ib][:, :],
                                start=(ib == 0), stop=(ib == 7))
                        nc.scalar.activation(
                            knt[j][:, sbi * 512:(sbi + 1) * 512], pk[:, :],
                            AF.Tanh, bias=bkt_sb[:, j:j + 1])

            # ---------------- phase Q ---------------------------------------
            with tc.tile_pool(name="qprj", bufs=4, space="PSUM") as kqprj:
                if True:
                    for sbi in range(2):
                        qin_t = []
                        for ib in range(8):
                            t = inp.tile([P, 512], BF16, tag=f"in{ib}", name=f"qin{ib}_{sbi}")
                            nc.sync.dma_start(
                                out=t[:, :],
                                in_=qt_in[ib * P:(ib + 1) * P, sbi * 512:(sbi + 1) * 512])
                            qin_t.append(t)
                        for j in range(NJ):
                            pq = kqprj.tile([P, 512], F32, tag="prj", name=f"pq{sbi}_{j}")
                            for ib in range(8):
                                nc.tensor.matmul(
                                    pq[:, :],
                                    lhsT=wq_t[ib][:, j * P:(j + 1) * P],
                                    rhs=qin_t[ib][:, :],
                                    start=(ib == 0), stop=(ib == 7))
                            nc.scalar.activation(
                                qn[j][:, sbi * 512:(sbi + 1) * 512], pq[:, :],
                                AF.Tanh, bias=bqt_sb[:, j:j + 1])
                    # fold 1/(sqrt(DK)*temp_h) into qn
                    for j in range(NJ):
                        nc.vector.tensor_scalar_mul(qn[j][:, :], qn[j][:, :],
                                                    tscp_sb[:, j:j + 1])

            eskq.close()   # free the K/Q input pool

            # ---------------- V proj + attention (overlapped) ---------------
            ctx = []
            esa = contextlib.ExitStack()
            maskp = esa.enter_context(tc.tile_pool(name="maskp", bufs=1))
            ppool = esa.enter_context(tc.tile_pool(name="pp", bufs=4))
            psc = esa.enter_context(tc.tile_pool(name="psc", bufs=2, space="PSUM"))

            esp1 = contextlib.ExitStack()
            pss = esp1.enter_context(tc.tile_pool(name="pss1", bufs=1,
                                                  space="PSUM"))

            esv = contextlib.ExitStack()
            vprj = esv.enter_context(tc.tile_pool(name="vprj", bufs=2, space="PSUM"))

            # wv and vin chunk 0 were prefetched during the K phase; mask
            # DMAs go out now
            mask_t = []
            for kt in range(NKT):
                t = maskp.tile([P, SQ], BF16, tag=f"m{kt}", name=f"mask{kt}")
                nc.sync.dma_start(out=t[:, :], in_=maskt[kt * P:(kt + 1) * P, :])
                mask_t.append(t)

            ctx_ps_cur = [None]      # ctx psum pair for the (j, qb) in flight

            def attn_open(j, qb):
                ctx_ps_cur[0] = [psc.tile([DK + 1, 512], F32, tag="ctx_ps",
                                          name=f"ctxps{j}_{qb}_{h2}", bufs=2)
                                 for h2 in range(2)]

            def attn_steps(pool, tag, sbufs, ptag, pbufs, j, qb, kts):
                """One scores-psum tile covering `kts` (1 or 2 k-tiles), one
                exp over the whole tile, then per-kt mask + ctx matmuls."""
                ctx_ps = ctx_ps_cur[0]
                W = SQ * len(kts)
                ps_s = pool.tile([P, W], F32, tag=tag,
                                 name=f"{tag}_{j}_{qb}_{kts[0]}", bufs=sbufs)
                for i, kt in enumerate(kts):
                    for h2 in range(2):
                        nc.tensor.matmul(
                            ps_s[:, i * SQ + h2 * 512:i * SQ + (h2 + 1) * 512],
                            lhsT=knt[j][h2 * DK:(h2 + 1) * DK,
                                        kt * P:(kt + 1) * P],
                            rhs=qn[j][h2 * DK:(h2 + 1) * DK,
                                      qb * 512:(qb + 1) * 512],
                            start=True, stop=True)
                p_t = ppool.tile([P, W], BF16, tag=ptag,
                                 name=f"p{ptag}_{j}_{qb}_{kts[0]}", bufs=pbufs)
                nc.scalar.activation(p_t[:, :], ps_s[:, :], AF.Exp)
                for i, kt in enumerate(kts):
                    nc.vector.tensor_tensor(
                        p_t[:, i * SQ:(i + 1) * SQ]
                            .rearrange("p (a b) -> p a b", a=2),
                        p_t[:, i * SQ:(i + 1) * SQ]
                            .rearrange("p (a b) -> p a b", a=2),
                        mask_t[kt][:, None, qb * 512:(qb + 1) * 512]
                            .to_broadcast((P, 2, 512)),
                        op=OP.mult)
                for i, kt in enumerate(kts):
                    for h2 in range(2):
                        h = 2 * j + h2
                        nc.tensor.matmul(
                            ctx_ps[h2][:, :],
                            lhsT=vau[h // 8][kt][:, h % 8, :],
                            rhs=p_t[:, i * SQ + h2 * 512:i * SQ + (h2 + 1) * 512],
                            start=(kt == 0), stop=(kt == NKT - 1))

            def attn_step(j, qb, kt):
                attn_steps(pss, "ps_s", 2, "p", 4, j, qb, [kt])

            def attn_close(j, qb):
                ctx_ps = ctx_ps_cur[0]
                if qb == 0:
                    ctx_j = qcp.tile([P, SQ], BF16, tag="qc", name=f"ctx{j}")
                    ctx.append(ctx_j)
                ctx_j = ctx[j]
                for h2 in range(2):
                    h = 2 * j + h2
                    nc.vector.tensor_scalar(
                        ctx_j[h2 * DK:(h2 + 1) * DK, qb * 512:(qb + 1) * 512],
                        ctx_ps[h2][0:DK, :], 1.0, None, op0=OP.mult)
                    # softmax sums rode the ctx matmul (ones column); stage to
                    # SBUF (DMA can't read PSUM), reshape through DRAM so the
                    # reciprocal runs 128 lanes wide (a [1,512] reciprocal
                    # monopolizes one DVE lane for ~3.4us and stalls the pipe)
                    sstage = ppool.tile([1, 512], F32, tag="sstage",
                                        name=f"sst{j}_{qb}_{h2}", bufs=2)
                    nc.vector.tensor_scalar(sstage[0:1, :],
                                            ctx_ps[h2][DK:DK + 1, :],
                                            1.0, None, op0=OP.mult)
                    nc.sync.dma_start(
                        out=sums_d[h:h + 1, qb * 512:(qb + 1) * 512],
                        in_=sstage[0:1, :])
                    # invert this qb's sums right away (128-wide via DRAM
                    # reshape) so the qb=1 close only assembles + multiplies
                    srow = ppool.tile([P, 4], F32, tag="srow",
                                      name=f"srow{j}_{qb}_{h2}", bufs=2)
                    nc.sync.dma_start(
                        out=srow[:, :],
                        in_=sums_d[h, qb * 512:(qb + 1) * 512]
                            .rearrange("(p c) -> p c", p=P))
                    nc.vector.reciprocal(srow[:, :], srow[:, :])
                    nc.sync.dma_start(
                        out=recip_d[h, qb * 512:(qb + 1) * 512]
                            .rearrange("(p c) -> p c", p=P),
                        in_=srow[:, :])
                if qb == 1:
                    bc = ppool.tile([P, SQ], F32, tag="bc", name=f"bc{j}", bufs=1)
                    nc.sync.dma_start(
                        out=bc[0:DK, :],
                        in_=recip_d[2 * j:2 * j + 1, :].to_broadcast((DK, SQ)))
                    nc.sync.dma_start(
                        out=bc[DK:P, :],
                        in_=recip_d[2 * j + 1:2 * j + 2, :].to_broadcast((DK, SQ)))
                    nc.vector.scalar_tensor_tensor(
                        ctx[j][:, :], ctx[j][:, :], 1.0, bc[:, :],
                        op0=OP.mult, op1=OP.mult)

            def v_group(st, ob):
                vin_t = vin_cur[0]
                str_ = st % 4
                pv = vprj.tile([P, 512], F32, tag="vprj", name=f"pv{st}_{ob}")
                for ib in range(8):
                    nc.tensor.matmul(
                        pv[:, :],
                        lhsT=vin_t[ib][:, str_ * P:(str_ + 1) * P],
                        rhs=wv_t[ib][:, ob * 512:(ob + 1) * 512],
                        start=(ib == 0), stop=False)
                nc.tensor.matmul(
                    pv[:, :], lhsT=onesr_sb[:, :],
                    rhs=bvr_sb[:, ob * 512:(ob + 1) * 512],
                    start=False, stop=True)
                nc.vector.tensor_scalar(
                    vau[ob][st][:, :, 0:DK],
                    pv[:, :].rearrange("p (a b) -> p a b", a=8),
                    1.0, None, op0=OP.mult)

            # V pass A (head-half ob0, feeds j=0..3) paced 1:1 with attention
            # steps of (j=0, qb=0)
            attn_open(0, 0)
            for st in range(NKT):
                if st % 4 == 0 and st > 0:
                    v_dma_chunk(st // 4, "A")
                v_group(st, 0)
                attn_step(0, 0, st)
            attn_close(0, 0)

            rmsw_sb = wrk.tile([P, D], BF16, name="rmsw_sb")
            nc.sync.dma_start(out=rmsw_sb[:, :], in_=rmsw[:, :])
            eps_t = wrk.tile([P, 1], F32, name="eps_t")
            nc.vector.memset(eps_t[:, :], EPS)

            # Attention j=0 (qb=1) then j=1..3: Act-saturated; V pass B (ob1,
            # feeds j=4..7) rides in the PE slack, one group every ~7 steps.
            vb_jobs = list(range(NKT))   # pass-B st groups still to emit
            groups_a = [(0, 1)] + [(j, qb) for j in range(1, 4) for qb in range(2)]
            nsteps = len(groups_a) * NKT
            placed = 0
            step_i = 0
            for (j, qb) in groups_a:
                attn_open(j, qb)
                for kt in range(NKT):
                    want = ((step_i + 1) * NKT) // nsteps
                    while placed < want:
                        st = vb_jobs[placed]
                        if st % 4 == 0:
                            v_dma_chunk(st // 4, "B")
                        v_group(st, 1)
                        placed += 1
                    attn_step(j, qb, kt)
                    step_i += 1
                attn_close(j, qb)
            while placed < NKT:
                st = vb_jobs[placed]
                if st % 4 == 0:
                    v_dma_chunk(st // 4, "B")
                v_group(st, 1)
                placed += 1

            esv.close()   # free the V psum

            # out-proj weight DMAs issue here (reusing the wv slots, whose
            # last reader was the final pass-B v_group just above)
            wo_t = []
            for ib in range(8):
                w_t = wop.tile([P, D], BF16, tag=f"wo{ib}", name=f"wo{ib}")
                nc.sync.dma_start(out=w_t[:, :], in_=wot[ib * P:(ib + 1) * P, :])
                wo_t.append(w_t)

            # attention j=4..7 (pure, Act-saturated)
            for j in range(4, NJ):
                for qb in range(2):
                    attn_open(j, qb)
                    for kt in range(NKT):
                        attn_step(j, qb, kt)
                    attn_close(j, qb)
            esp1.close()

            # kn / v_aug / mask / p no longer needed
            esa.close()
            esvin.close()
            es2.close()

            # ------------- out-proj + RMSNorm --------------------------
            with tc.tile_pool(name="outp", bufs=2) as outp, \
                 tc.tile_pool(name="scrp", bufs=2) as scrp, \
                 tc.tile_pool(name="pop", bufs=6, space="PSUM") as pop:
                for st in range(8):
                    o_sb = outp.tile([P, D], BF16, tag="o", name=f"o{st}")
                    for ob in range(2):
                        po = pop.tile([P, 512], F32, tag="po", name=f"po{st}_{ob}")
                        for db in range(8):
                            nc.tensor.matmul(
                                po[:, :],
                                lhsT=ctx[db][:, st * P:(st + 1) * P],
                                rhs=wo_t[db][:, ob * 512:(ob + 1) * 512],
                                start=(db == 0), stop=False)
                        nc.tensor.matmul(
                            po[:, :], lhsT=onesr_sb[:, :],
                            rhs=bor_sb[:, ob * 512:(ob + 1) * 512],
                            start=False, stop=True)
                        # Act does the PSUM evacuation: the DVE is the tail
                        # bottleneck, Act is idle here
                        nc.scalar.copy(o_sb[:, ob * 512:(ob + 1) * 512],
                                       po[:, :])
                    sq_t = scrp.tile([P, D], BF16, tag="sq", name=f"sq{st}")
                    ssq = scrp.tile([P, 1], F32, tag="ssq", name=f"ssq{st}")
                    nc.vector.scalar_tensor_tensor(
                        sq_t[:, :], o_sb[:, :], 1.0, o_sb[:, :],
                        op0=OP.mult, op1=OP.mult, accum_out=ssq[:, :])
                    rms1 = scrp.tile([P, 1], F32, tag="rms1", name=f"rms1{st}")
                    nc.scalar.activation(rms1[:, :], ssq[:, :], AF.Sqrt,
                                         bias=eps_t[:, :], scale=1.0 / D)
                    nc.vector.reciprocal(rms1[:, :], rms1[:, :])
                    o_f = outp.tile([P, D], F32, tag="of", name=f"of{st}")
                    nc.vector.scalar_tensor_tensor(
                        o_f[:, :], o_sb[:, :], rms1[:, :], rmsw_sb[:, :],
                        op0=OP.mult, op1=OP.mult)
                    nc.sync.dma_start(out=out[st * P:(st + 1) * P, :],
                                      in_=o_f[:, :])

    nc.compile()
    return nc


_NC_CACHE = []


def _get_nc():
    if not _NC_CACHE:
        _NC_CACHE.append(build_nc())
    return _NC_CACHE[0]


def _fuse_na(w, b, na_w, na_b):
    """Fold the per-head NeuralAttention transform into the projection.

    reference: tanh(split(X @ w.T + b) @ na_w.T + na_b)
             = tanh(split(X @ (BD@w).T + (BD@b + tile(na_b))))
    with BD = blockdiag(na_w) over the H heads.  Returns (w_f.T, b_f).
    """
    w64 = w.astype(np.float64)
    wf = np.empty((D, D), np.float64)
    bf = np.empty((D,), np.float64)
    na64 = na_w.astype(np.float64)
    for h in range(H):
        sl = slice(h * DK, (h + 1) * DK)
        wf[sl, :] = na64 @ w64[sl, :]
        bf[sl] = na64 @ b.astype(np.float64)[sl] + na_b.astype(np.float64)
    return (np.ascontiguousarray(wf.T.astype(np.float32)),
            bf.astype(np.float32))


def _prep_in_maps(Q, K, V, mask, wq, bq, wk, bk, wv, bv, wo, bo,
                  na_q_w, na_q_b, na_k_w, na_k_b, temperature, rms_w):
    f = lambda x: np.asarray(x, dtype=np.float32)
    Q, K, V = f(Q), f(K), f(V)
    mask = np.asarray(mask)

    wqt_f, bq_f = _fuse_na(f(wq), f(bq), f(na_q_w), f(na_q_b))
    wkt_f, bk_f = _fuse_na(f(wk), f(bk), f(na_k_w), f(na_k_b))

    shared = dict(
        wqt=wqt_f.astype(ml_dtypes.bfloat16),
        wkt=wkt_f.astype(ml_dtypes.bfloat16),
        wvt=np.ascontiguousarray(f(wv).T).astype(ml_dtypes.bfloat16),
        wot=np.ascontiguousarray(f(wo).T).astype(ml_dtypes.bfloat16),
        bqt=np.ascontiguousarray(bq_f.reshape(8, P).T),
        bkt=np.ascontiguousarray(bk_f.reshape(8, P).T),
        bvr=_f32r_round(f(bv).reshape(1, D)),
        bor=_f32r_round(f(bo).reshape(1, D)),
        rmsw=np.ascontiguousarray(
            np.broadcast_to(f(rms_w), (P, D))).astype(ml_dtypes.bfloat16),
        onesr=np.ones((1, P), np.float32),
    )
    ts = 1.0 / (np.sqrt(DK).astype(np.float32) * f(temperature).reshape(H))
    tscp = np.empty((P, 8), np.float32)
    for j in range(NJ):
        tscp[0:DK, j] = ts[2 * j]
        tscp[DK:P, j] = ts[2 * j + 1]
    shared["tscp"] = tscp

    kts, vts = {}, {}
    for b in range(B):
        kts[b] = np.ascontiguousarray(K[b].T).astype(ml_dtypes.bfloat16)
        vts[b] = np.ascontiguousarray(V[b].T).astype(ml_dtypes.bfloat16)

    in_maps = []
    for c in range(NCORES):
        b, hf = divmod(c, 2)
        qsl = slice(hf * SQ, (hf + 1) * SQ)
        m = dict(shared)
        m["qt_in"] = np.ascontiguousarray(Q[b, qsl, :].T).astype(ml_dtypes.bfloat16)
        m["kt_in"] = kts[b]
        m["vt_in"] = vts[b]
        m["maskt"] = np.ascontiguousarray(
            mask[b, 0, qsl, :].T).astype(ml_dtypes.bfloat16)
        in_maps.append(m)
    return in_maps


def _run(in_maps, **kwargs):
    nc = _get_nc()
    return run_bass_kernel_spmd(nc, in_maps, core_ids=list(range(NCORES)), **kwargs)


def kernel(**inputs):
    in_maps = _prep_in_maps(**inputs)
    res = _run(in_maps)
    out = np.empty((B, S, D), np.float32)
    for c in range(NCORES):
        b, hf = divmod(c, 2)
        out[b, hf * SQ:(hf + 1) * SQ, :] = res.results[c]["out"]
    return out
